# revision 1
# baseline (speedup 1.0000x reference)
"""Aitchison-Aitken categorical kernel on 8 TRN2 NeuronCores.

Math (reference, NUM_LEVELS=4, n_feat=64):
    w_f     = log(1-h_f) - log(h_f/3)
    base    = sum_f log(h_f/3) - sum_f log(h_f) = -64*log(3)   (data independent)
    match   = sum_f w_f * 1[test_if == train_jf]           ([n_test, n_train])
    ld      = match + base
    out     = rowmax(ld) * exp(ld - rowmax(ld))

Device algorithm (per core, data-parallel over test rows):
  - encode test/train as fp16 level-indicator matrices, K = 4*64 = 256
    (2 K-tiles of 128: [lvl0|lvl1], [lvl2|lvl3], feature-duplicated halves);
    weights w folded into the test-side encoding
  - feature duplication happens during the f32->f16 staging cast (GPSIMD),
    so each PE transpose is a full [128,128] block and the PSUM evacuation
    is a single straight copy (GPSIMD) -- no SBUF dup DMAs at all
  - one shared PSUM pool (2 bufs x 4 banks) serves both the encode
    transposes (f16 tiles) and the matmul blocks (f32 [128,2048] tiles)
  - per m-tile: 4 col-blocks of 2048; matmul stationary order A B|B A|A B|B A
    so the stationary reloads are minimized (4 flips/m instead of 16)
  - ebuf = exp(match + base) via ScalarE straight from PSUM -> bf16
  - row max of ebuf via a bf16 tensor_tensor max fold ladder (2x DVE mode)
  - c = ln(m)/m  (identity: m = e^maxld  =>  c*e^ld = maxld*e^{ld-maxld})
  - obuf = ebuf * c (DVE tensor_scalar, 4x mode), DMA'd per 2048-col chunk
    on alternating queues; last m-tile split finer to shrink the tail

Sharding: test_Xs rows across 8 cores; bandwidths/train_Xs replicated;
out [1024, 8192] local per core, host-concatenated.
"""
import numpy as np
from contextlib import ExitStack

from concourse import bacc, hw_specs, mybir, masks, tile
from concourse.bass_utils import run_bass_kernel_spmd

f32 = mybir.dt.float32
f16 = mybir.dt.float16
bf16 = mybir.dt.bfloat16
ACTF = mybir.ActivationFunctionType
ALU = mybir.AluOpType

N_CORES = 8
N_TEST, N_TRAIN, N_FEAT = 8192, 8192, 64
M_LOC = N_TEST // N_CORES          # 1024 test rows per core
P = 128                            # partitions
M_TILES = M_LOC // P               # 8
NT = 512                           # train cols per matmul
QCOL = 1024                        # psum block (2 banks f32)
NQ = N_TRAIN // QCOL               # 8
EBLK = 2048                        # encode block (f16 cols per prep tile)
BASE = float(-N_FEAT * np.log(3.0))


# ---------------------------------------------------------------------------
# Activation-table patch: both Exp and Ln are needed; the stock fixpoint pass
# resolves each to the first table set containing it (exp_and_others vs
# natural_log), reloading the ACT tables (~1.3us) on every switch. Hide
# Exp/Ln from every other set so both resolve to natural_log_exp_and_others
# at its true act_func_set_id (list order/indices preserved).
_COMBINED = "natural_log_exp_and_others"
_orig_get_tables = hw_specs.get_activation_tables


def _patched_tables(module_arch):
    d = _orig_get_tables(module_arch)
    if _COMBINED not in d:
        return d
    hide = {ACTF.Exp, ACTF.Ln}
    return {k: (set(v) if k == _COMBINED else set(v) - hide)
            for k, v in d.items()}


hw_specs.get_activation_tables = _patched_tables
bacc.get_activation_tables = _patched_tables   # bacc imported its own ref
# ---------------------------------------------------------------------------


def _build():
    nc = bacc.Bacc(None, target_bir_lowering=False)
    bw_ext = nc.declare_dram_parameter("bandwidths", [N_FEAT], f32, isOutput=False)
    test_ext = nc.declare_dram_parameter("test_Xs", [M_LOC, N_FEAT], f32, isOutput=False)
    train_ext = nc.declare_dram_parameter("train_Xs", [N_TRAIN, N_FEAT], f32, isOutput=False)
    out_ext = nc.declare_dram_parameter("out", [M_LOC, N_TRAIN], f16, isOutput=True)

    hwq = [nc.sync, nc.scalar]  # the two HWDGE queues

    with tile.TileContext(nc) as tc, ExitStack() as ctx:
        const = ctx.enter_context(tc.tile_pool(name="const", bufs=1))
        ps = ctx.enter_context(tc.tile_pool(name="ps", bufs=4, space="PSUM"))
        enc = ctx.enter_context(tc.tile_pool(name="enc", bufs=1))
        stage_p = ctx.enter_context(tc.tile_pool(name="stage", bufs=3))
        nat_p = ctx.enter_context(tc.tile_pool(name="nat", bufs=3))
        stats = ctx.enter_context(tc.tile_pool(name="stats", bufs=4))
        fold_p = ctx.enter_context(tc.tile_pool(name="fold", bufs=2))
        ebuf_pool = ctx.enter_context(tc.tile_pool(name="ebuf", bufs=2))
        out_pool = ctx.enter_context(tc.tile_pool(name="obuf", bufs=4))

        # ---- bw + test stage DMA first (sync HWDGE, no Q7 startup);
        # memsets + w-prep Lns next so the implicit ACT table load runs
        # on the scalar queue before anything else ------------------------
        bw = const.tile([N_FEAT, 1], f32)
        nc.sync.dma_start(out=bw[:], in_=bw_ext[:].rearrange("(f o) -> f o", o=1))
        N_GRP_T = M_LOC // 1024            # 1
        N_GRP_S = N_TRAIN // 1024          # 8
        stage_tiles = []

        def stage_group(g):
            if g < N_GRP_T:
                src, row0 = test_ext, g * 1024
            else:
                src, row0 = train_ext, (g - N_GRP_T) * 1024
            st = stage_p.tile([P, 512], f32, tag="stage")
            nc.sync.dma_start(
                out=st[:].rearrange("p (c f) -> p c f", f=N_FEAT),
                in_=src[row0:row0 + 1024, :].rearrange("(c p) f -> p c f", p=P))
            stage_tiles.append(st)

        stage_group(0)                     # test

        one_t = const.tile([N_FEAT, 1], f32)
        nc.vector.memset(one_t[:], 1.0)
        base_t = const.tile([P, 1], f32)
        nc.vector.memset(base_t[:], BASE)
        # dummy activation: triggers the combined Exp/Ln table load now,
        # off the critical path (the load would otherwise delay the w-prep)
        scratch = const.tile([P, 1], f32)
        nc.scalar.activation(scratch[:], base_t[:], ACTF.Exp)
        lvlA = const.tile([P, 1], f32)
        nc.vector.memset(lvlA[0:64, :], 0.0)
        nc.vector.memset(lvlA[64:128, :], 1.0)
        lvlB = const.tile([P, 1], f32)
        nc.vector.memset(lvlB[0:64, :], 2.0)
        nc.vector.memset(lvlB[64:128, :], 3.0)

        # ---- w vector from bandwidths ----------------------------------
        lt = const.tile([N_FEAT, 1], f32)   # log(1 - h)
        nc.scalar.activation(lt[:], bw[:], ACTF.Ln, bias=one_t[:], scale=-1.0)
        lf = const.tile([N_FEAT, 1], f32)   # log(h/3)
        nc.scalar.activation(lf[:], bw[:], ACTF.Ln, scale=1.0 / 3.0)
        w2 = const.tile([P, 1], f32)
        nc.vector.tensor_tensor(w2[0:64, :], lt[:], lf[:], op=ALU.subtract)
        nc.vector.tensor_copy(w2[64:128, :], w2[0:64, :])

        ident16 = const.tile([P, P], f16)
        masks.make_identity(nc, ident16[:])

        # train stage DMAs after the w-prep
        for g in range(1, N_GRP_T + N_GRP_S):
            stage_group(g)

        # ---- encode: cast+dup (GPSIMD), transpose (PE), evac (GPSIMD),
        #      level indicators (DVE 4x) --------------------------------
        tencA = enc.tile([P, M_LOC], f16)
        tencB = enc.tile([P, M_LOC], f16)
        sencA = enc.tile([P, N_TRAIN], f16)
        sencB = enc.tile([P, N_TRAIN], f16)
        dstT_t = enc.tile([P, M_LOC], f16)
        dstT_s = enc.tile([P, N_TRAIN], f16)

        def cast_dup(g):
            # f32->f16 cast via a software-DGE DMA (gpsimd issues, DMA
            # engines convert in flight -- zero compute-engine time), then
            # nat16[p, c*128 + d*64 + f] = natc[p, c*64 + f] via 4x DVE copy
            st = stage_tiles[g]
            natc = nat_p.tile([P, 512], f16, tag="natc")
            nc.gpsimd.dma_start(out=natc[:], in_=st[:])
            nat = nat_p.tile([P, 1024], f16, tag="nat")
            natv = nat[:].rearrange("p (c d) -> p c d", d=2 * N_FEAT)
            ncv = natc[:].rearrange("p (c f) -> p c f", f=N_FEAT)
            for d in range(2):
                nc.vector.tensor_copy(natv[:, :, d * N_FEAT:(d + 1) * N_FEAT], ncv)
            return nat

        def transpose_block(nat_tiles, dstT, col0, ncols, evac_eng):
            # one psum prep tile covering `ncols` transposed cols; shares the
            # "mm" tag (and thus the 4x2-bank ring) with the matmul blocks
            ptf = ps.tile([P, QCOL], f32, tag="mm")
            pt = ptf[:, 0:ncols // 2].bitcast(f16)
            for j in range(ncols // P):
                gi, ci = divmod(col0 // P + j, 8)
                nat = nat_tiles[gi]
                nc.tensor.transpose(pt[:, j * P:(j + 1) * P],
                                    nat[:, ci * P:(ci + 1) * P], ident16[:])
            if evac_eng is nc.scalar:
                nc.scalar.activation(dstT[:, col0:col0 + ncols], pt[:],
                                     ACTF.Copy, bias=0.0, scale=1.0)
            else:
                nc.vector.tensor_copy(dstT[:, col0:col0 + ncols], pt[:])

        def is_equal_block(dstT, dstA, dstB, col0, ncols, wmul):
            s = slice(col0, col0 + ncols)
            if wmul is None:
                nc.vector.tensor_scalar(dstA[:, s], dstT[:, s], lvlA[:], None,
                                        op0=ALU.is_equal)
                nc.vector.tensor_scalar(dstB[:, s], dstT[:, s], lvlB[:], None,
                                        op0=ALU.is_equal)
            else:
                nc.vector.tensor_scalar(dstA[:, s], dstT[:, s], lvlA[:], wmul[:],
                                        op0=ALU.is_equal, op1=ALU.mult)
                nc.vector.tensor_scalar(dstB[:, s], dstT[:, s], lvlB[:], wmul[:],
                                        op0=ALU.is_equal, op1=ALU.mult)

        # test: 1 group -> 1 prep tile of 1024 cols (evac on ACT)
        nat_t = [cast_dup(0)]
        transpose_block(nat_t, dstT_t, 0, 1024, nc.scalar)
        is_equal_block(dstT_t, tencA, tencB, 0, M_LOC, w2)

        # train: 8 groups -> 4 prep tiles of 2048 cols.  Blocks 0/1 are
        # emitted up front (evac on ACT, which is idle pre-loop); blocks
        # 2/3 are interleaved into m=0 (evac on ACT/DVE) so the psum ring
        # and the engine queues stay deadlock-free and pipelined.
        nat_s = {}

        def encode_train_block(t, evac_eng):
            nat_s[2 * t] = cast_dup(1 + 2 * t)
            nat_s[2 * t + 1] = cast_dup(1 + 2 * t + 1)
            transpose_block(nat_s, dstT_s, t * EBLK, EBLK, evac_eng)
            is_equal_block(dstT_s, sencA, sencB, t * EBLK, EBLK, None)

        encode_train_block(0, nc.scalar)

        # ---- main loop: 4 pairs of 1024-col psum tiles per m-tile -------
        for m in range(M_TILES):
            ms = slice(m * P, (m + 1) * P)
            ebuf = ebuf_pool.tile([P, N_TRAIN], bf16)
            fas = [fold_p.tile([P, QCOL], bf16, tag=f"fa{k}", name=f"fa{k}")
                   for k in range(4)]
            g1 = fold_p.tile([P, QCOL], bf16, tag="g1")
            g2 = fold_p.tile([P, QCOL], bf16, tag="g2")
            g3 = fold_p.tile([P, QCOL], bf16, tag="g3")
            prs = []
            for pair in range(4):
                if m == 0 and pair < 3:
                    encode_train_block(pair + 1,
                                       nc.scalar if pair < 2 else nc.vector)
                pst0 = ps.tile([P, QCOL], f32, tag="mm")
                pst1 = ps.tile([P, QCOL], f32, tag="mm")
                # stationary-reuse: even pair -> A then B, odd -> B then A;
                # each stationary serves both tiles (4 matmuls per load)
                first, second = ((tencA, sencA), (tencB, sencB))
                if pair % 2 == 1:
                    first, second = second, first
                for (tenc, senc), start in ((first, True), (second, False)):
                    for ti, pst in ((0, pst0), (1, pst1)):
                        for j in range(QCOL // NT):
                            n = (2 * pair + ti) * (QCOL // NT) + j
                            nc.tensor.matmul(pst[:, j * NT:(j + 1) * NT],
                                             tenc[:, ms],
                                             senc[:, n * NT:(n + 1) * NT],
                                             start=start, stop=not start)
                for ti, pst in ((0, pst0), (1, pst1)):
                    q = 2 * pair + ti
                    qs = slice(q * QCOL, (q + 1) * QCOL)
                    nc.scalar.activation(ebuf[:, qs], pst[:], ACTF.Exp,
                                         bias=base_t[:], scale=1.0)
                # first-level max fold for this pair (DVE, 2x bf16)
                nc.vector.tensor_tensor(
                    fas[pair][:], ebuf[:, 2 * pair * QCOL:(2 * pair + 1) * QCOL],
                    ebuf[:, (2 * pair + 1) * QCOL:(2 * pair + 2) * QCOL],
                    op=ALU.max)
                if m == M_TILES - 1:
                    # last m-tile: reduce each pair immediately so the tail
                    # chain after the final exp is as short as possible
                    pr = stats.tile([P, 1], f32, tag=f"pr{pair}",
                                    name=f"pr{pair}")
                    nc.vector.tensor_reduce(pr[:], fas[pair][:],
                                            axis=mybir.AxisListType.X,
                                            op=ALU.max)
                    prs.append(pr)
                elif pair == 1:
                    nc.vector.tensor_tensor(g1[:], fas[0][:], fas[1][:],
                                            op=ALU.max)
            mm_t = stats.tile([P, 1], f32)     # m = max(ebuf) (exact in bf16)
            if m == M_TILES - 1:
                pra = stats.tile([P, 1], f32)
                prb = stats.tile([P, 1], f32)
                nc.vector.tensor_tensor(pra[:], prs[0][:], prs[1][:], op=ALU.max)
                nc.vector.tensor_tensor(prb[:], prs[2][:], prs[3][:], op=ALU.max)
                nc.vector.tensor_tensor(mm_t[:], pra[:], prb[:], op=ALU.max)
            else:
                # fold ladder tail on DVE (2x bf16 TT mode)
                nc.vector.tensor_tensor(g2[:], fas[2][:], fas[3][:], op=ALU.max)
                nc.vector.tensor_tensor(g3[:], g1[:], g2[:], op=ALU.max)
                nc.vector.tensor_tensor(g2[:, 0:512], g3[:, 0:512],
                                        g3[:, 512:1024], op=ALU.max)
                nc.vector.tensor_tensor(g3[:, 0:256], g2[:, 0:256],
                                        g2[:, 256:512], op=ALU.max)
                nc.vector.tensor_reduce(mm_t[:], g3[:, 0:256],
                                        axis=mybir.AxisListType.X, op=ALU.max)
            # c = ln(m)/m
            lnm = stats.tile([P, 1], f32)
            nc.scalar.activation(lnm[:], mm_t[:], ACTF.Ln)
            rec = stats.tile([P, 1], f32)
            nc.vector.reciprocal(rec[:], mm_t[:])
            cvec = stats.tile([P, 1], f32)
            nc.vector.tensor_tensor(cvec[:], lnm[:], rec[:], op=ALU.mult)

            for h in range(4):
                hs = slice(h * EBLK, (h + 1) * EBLK)
                ob = out_pool.tile([P, EBLK], f16, tag="ob")
                if m == M_TILES - 1 and h % 2 == 1:
                    # last m-tile: ACT is idle after the final exp; split the
                    # mults across both engines to halve the tail
                    nc.scalar.activation(ob[:], ebuf[:, hs], ACTF.Copy,
                                         bias=0.0, scale=cvec[:])
                else:
                    nc.vector.tensor_scalar(ob[:], ebuf[:, hs], cvec[:],
                                            None, op0=ALU.mult)
                nc.sync.dma_start(out=out_ext[ms, hs], in_=ob[:])

    nc.compile()
    return nc


_NC = None


def _get_nc():
    global _NC
    if _NC is None:
        _NC = _build()
    return _NC


def kernel(bandwidths, test_Xs, train_Xs):
    bandwidths = np.ascontiguousarray(bandwidths, dtype=np.float32)
    test_Xs = np.ascontiguousarray(test_Xs, dtype=np.float32)
    train_Xs = np.ascontiguousarray(train_Xs, dtype=np.float32)

    nc = _get_nc()
    in_maps = [
        {
            "bandwidths": bandwidths,
            "test_Xs": np.ascontiguousarray(test_Xs[i * M_LOC:(i + 1) * M_LOC]),
            "train_Xs": train_Xs,
        }
        for i in range(N_CORES)
    ]
    res = run_bass_kernel_spmd(nc, in_maps, core_ids=list(range(N_CORES)))
    return np.concatenate([np.asarray(r["out"]).astype(np.float32)
                           for r in res.results], axis=0)


if __name__ == "__main__":
    rng = np.random.default_rng(0)
    h = rng.uniform(0.05, 0.5, N_FEAT).astype(np.float32)
    t = rng.integers(0, 4, (N_TEST, N_FEAT)).astype(np.float32)
    s = rng.integers(0, 4, (N_TRAIN, N_FEAT)).astype(np.float32)
    out = kernel(bandwidths=h, test_Xs=t, train_Xs=s)
    print(out.shape, out.dtype)



# revision 9
# speedup vs baseline: 1.1541x; 1.1541x over previous
"""Aitchison-Aitken categorical kernel on 8 TRN2 NeuronCores.

Math (reference, NUM_LEVELS=4, n_feat=64):
    w_f     = log(1-h_f) - log(h_f/3)
    base    = sum_f log(h_f/3) - sum_f log(h_f) = -64*log(3)   (data independent)
    match   = sum_f w_f * 1[test_if == train_jf]           ([n_test, n_train])
    ld      = match + base
    out     = rowmax(ld) * exp(ld - rowmax(ld))
          ( = c * exp(ld) with c = ln(m)/m, m = e^{rowmax ld} )

Device algorithm (per core, data-parallel over test rows):
  - host pre-lays-out test/train as feature-major f16 with the 64 features
    duplicated to 128 partitions ([dup | dup]); the device then needs no
    transposes, no PSUM encode staging, and only ~300 DMA descriptors
  - level-indicator encodings A=[lvl0|lvl1], B=[lvl2|lvl3] via DVE
    tensor_scalar is_equal (w folded into the test side), K = 2 x 128
  - per m-tile (128 test rows): 4 psum blocks of [128,2048] f32 (4 banks,
    ring of 2), each = 4 matmuls of 1024 cols (A,A,B,B accumulate)
  - exp(match + base) via one ScalarE activation per 2048 block,
    PSUM -> bf16 ebuf (bf16: e^ld spans e^-157..e^+41)
  - row max via two fused tensor_tensor_reduce (pairwise max of ebuf
    blocks + running reduce, init chained) on DVE
  - c = ln(m)/m; obuf = ebuf * c (DVE tensor_scalar 4x); DMA per 2048 cols
  - ln(m) for m-tile m is emitted between m+1's exps to avoid ACT bubbles

Sharding: test_Xs rows across 8 cores; bandwidths/train_Xs replicated;
out [1024, 8192] f16 local per core, host-concatenated.
"""
import numpy as np
from contextlib import ExitStack

from concourse import bacc, hw_specs, mybir, tile
from concourse.bass_utils import run_bass_kernel_spmd

f32 = mybir.dt.float32
f16 = mybir.dt.float16
bf16 = mybir.dt.bfloat16
ACTF = mybir.ActivationFunctionType
ALU = mybir.AluOpType

N_CORES = 8
N_TEST, N_TRAIN, N_FEAT = 8192, 8192, 64
M_LOC = N_TEST // N_CORES          # 1024 test rows per core
P = 128                            # partitions
M_TILES = M_LOC // P               # 8
QCOL = 2048                        # psum block (4 banks f32)
NQ = 4                             # blocks per m-tile
NMM = 512                          # cols per matmul (one PSUM bank)
EXP_WIDE = False                   # exp over 2048 (4-bank) vs 2x1024 PSUM reads
USE_TTR = False                    # fused tensor_tensor_reduce for row max
BASE = float(-N_FEAT * np.log(3.0))
NEG_INF = float(np.float32(-3.0e38))


# ---------------------------------------------------------------------------
# Activation-table patch: both Exp and Ln are needed; the stock fixpoint pass
# resolves each to the first table set containing it (exp_and_others vs
# natural_log), reloading the ACT tables (~1.3us) on every switch. Hide
# Exp/Ln from every other set so both resolve to natural_log_exp_and_others
# at its true act_func_set_id (list order/indices preserved).
_COMBINED = "natural_log_exp_and_others"
_orig_get_tables = hw_specs.get_activation_tables


def _patched_tables(module_arch):
    d = _orig_get_tables(module_arch)
    if _COMBINED not in d:
        return d
    hide = {ACTF.Exp, ACTF.Ln}
    return {k: (set(v) if k == _COMBINED else set(v) - hide)
            for k, v in d.items()}


hw_specs.get_activation_tables = _patched_tables
bacc.get_activation_tables = _patched_tables   # bacc imported its own ref
# ---------------------------------------------------------------------------


def _build():
    nc = bacc.Bacc(None, target_bir_lowering=False)
    bw_ext = nc.declare_dram_parameter("bandwidths", [N_FEAT], f32, isOutput=False)
    # host-prepped: feature-major f16, features duplicated across halves
    test_ext = nc.declare_dram_parameter("testT", [P, M_LOC], f16, isOutput=False)
    train_ext = nc.declare_dram_parameter("trainT", [P, N_TRAIN], f16, isOutput=False)
    out_ext = nc.declare_dram_parameter("out", [M_LOC, N_TRAIN], f16, isOutput=True)

    with tile.TileContext(nc) as tc, ExitStack() as ctx:
        const = ctx.enter_context(tc.tile_pool(name="const", bufs=1))
        ps = ctx.enter_context(tc.tile_pool(name="ps", bufs=2, space="PSUM"))
        enc = ctx.enter_context(tc.tile_pool(name="enc", bufs=1))
        stats = ctx.enter_context(tc.tile_pool(name="stats", bufs=4))
        scr = ctx.enter_context(tc.tile_pool(name="scr", bufs=2))
        ebuf_pool = ctx.enter_context(tc.tile_pool(name="ebuf", bufs=2))
        out_pool = ctx.enter_context(tc.tile_pool(name="obuf", bufs=4))

        # ---- input DMAs first (sync HWDGE), then memsets + the dummy
        # activation so the combined Exp/Ln table load runs immediately ----
        bw = const.tile([N_FEAT, 1], f32)
        nc.sync.dma_start(out=bw[:], in_=bw_ext[:].rearrange("(f o) -> f o", o=1))
        dstT_t = enc.tile([P, M_LOC], f16)
        nc.sync.dma_start(out=dstT_t[:], in_=test_ext[:])
        dstT_s = enc.tile([P, N_TRAIN], f16)
        NCH = 4
        CH = N_TRAIN // NCH
        for c in range(NCH):
            nc.sync.dma_start(out=dstT_s[:, c * CH:(c + 1) * CH],
                              in_=train_ext[:, c * CH:(c + 1) * CH])

        one_t = const.tile([N_FEAT, 1], f32)
        nc.vector.memset(one_t[:], 1.0)
        base_t = const.tile([P, 1], f32)
        nc.vector.memset(base_t[:], BASE)
        # dummy activation: triggers the combined Exp/Ln table load now
        scratch = const.tile([P, 1], f32)
        nc.scalar.activation(scratch[:], base_t[:], ACTF.Exp)
        lvlA = const.tile([P, 1], f32)
        nc.vector.memset(lvlA[0:64, :], 0.0)
        nc.vector.memset(lvlA[64:128, :], 1.0)
        lvlB = const.tile([P, 1], f32)
        nc.vector.memset(lvlB[0:64, :], 2.0)
        nc.vector.memset(lvlB[64:128, :], 3.0)
        ninf_t = const.tile([P, 1], f32)
        nc.vector.memset(ninf_t[:], NEG_INF)

        # ---- w vector from bandwidths ----------------------------------
        lt = const.tile([N_FEAT, 1], f32)   # log(1 - h)
        nc.scalar.activation(lt[:], bw[:], ACTF.Ln, bias=one_t[:], scale=-1.0)
        lf = const.tile([N_FEAT, 1], f32)   # log(h/3)
        nc.scalar.activation(lf[:], bw[:], ACTF.Ln, scale=1.0 / 3.0)
        w2 = const.tile([P, 1], f32)
        nc.vector.tensor_tensor(w2[0:64, :], lt[:], lf[:], op=ALU.subtract)
        nc.vector.tensor_copy(w2[64:128, :], w2[0:64, :])

        # ---- encodings: is_equal (DVE), w folded into test side --------
        tencA = enc.tile([P, M_LOC], f16)
        tencB = enc.tile([P, M_LOC], f16)
        sencA = enc.tile([P, N_TRAIN], f16)
        sencB = enc.tile([P, N_TRAIN], f16)
        nc.vector.tensor_scalar(tencA[:], dstT_t[:], lvlA[:], w2[:],
                                op0=ALU.is_equal, op1=ALU.mult)
        nc.vector.tensor_scalar(tencB[:], dstT_t[:], lvlB[:], w2[:],
                                op0=ALU.is_equal, op1=ALU.mult)
        for c in range(NCH):
            cs = slice(c * CH, (c + 1) * CH)
            nc.vector.tensor_scalar(sencA[:, cs], dstT_s[:, cs], lvlA[:], None,
                                    op0=ALU.is_equal)
            nc.vector.tensor_scalar(sencB[:, cs], dstT_s[:, cs], lvlB[:], None,
                                    op0=ALU.is_equal)

        # ---- main loop --------------------------------------------------
        # The finish chain of m-tile m (ln -> recip -> cvec -> mults ->
        # DMAs) is deferred and emitted between m+1's exp q1 and q2.  By
        # then m's DVE max-reduce is long done, so ACT reaches the Ln with
        # no wait and the queue never bubbles; m's output mults run on DVE
        # while ACT exps m+1's later blocks.
        pending = [None]

        def run_mtile(m):
            ms = slice(m * P, (m + 1) * P)
            last = m == M_TILES - 1
            ebuf = ebuf_pool.tile([P, N_TRAIN], bf16)
            for q in range(NQ):
                qs = slice(q * QCOL, (q + 1) * QCOL)
                pst = ps.tile([P, QCOL], f32, tag="mm")
                # A,A,B,B per block; alternate order across blocks so the
                # boundary stationary is shared (ldweights pull-ahead hides
                # the rest)
                order = ((tencA, sencA), (tencB, sencB))
                if q % 2 == 1:
                    order = (order[1], order[0])
                (t0, s0), (t1, s1) = order
                for j in range(QCOL // NMM):
                    js = slice(q * QCOL + j * NMM, q * QCOL + (j + 1) * NMM)
                    nc.tensor.matmul(pst[:, j * NMM:(j + 1) * NMM],
                                     t0[:, ms], s0[:, js],
                                     start=True, stop=False)
                for j in range(QCOL // NMM):
                    js = slice(q * QCOL + j * NMM, q * QCOL + (j + 1) * NMM)
                    nc.tensor.matmul(pst[:, j * NMM:(j + 1) * NMM],
                                     t1[:, ms], s1[:, js],
                                     start=False, stop=True)
                if EXP_WIDE:
                    nc.scalar.activation(ebuf[:, qs], pst[:], ACTF.Exp,
                                         bias=base_t[:], scale=1.0)
                else:
                    for e in range(2):
                        nc.scalar.activation(
                            ebuf[:, q * QCOL + e * 1024:q * QCOL + (e + 1) * 1024],
                            pst[:, e * 1024:(e + 1) * 1024], ACTF.Exp,
                            bias=base_t[:], scale=1.0)
                if q == 1 and pending[0] is not None:
                    pending[0]()
                    pending[0] = None

            # row max over ebuf
            mm_t = stats.tile([P, 1], f32, tag="mm_t", name="mm_t")
            if USE_TTR:
                # two fused pairwise-max+reduce, init chained
                sc01 = scr.tile([P, QCOL], bf16, tag="sc")
                sc23 = scr.tile([P, QCOL], bf16, tag="sc")
                bm01 = stats.tile([P, 1], f32, tag="bm01", name="bm01")
                nc.vector.tensor_tensor_reduce(
                    sc01[:], ebuf[:, 0:QCOL], ebuf[:, QCOL:2 * QCOL], 1.0,
                    ninf_t[:], op0=ALU.max, op1=ALU.max, accum_out=bm01[:])
                nc.vector.tensor_tensor_reduce(
                    sc23[:], ebuf[:, 2 * QCOL:3 * QCOL],
                    ebuf[:, 3 * QCOL:4 * QCOL],
                    1.0, bm01[:], op0=ALU.max, op1=ALU.max, accum_out=mm_t[:])
            else:
                g1 = scr.tile([P, QCOL], bf16, tag="sc")
                g2 = scr.tile([P, QCOL], bf16, tag="sc")
                nc.vector.tensor_tensor(g1[:], ebuf[:, 0:QCOL],
                                        ebuf[:, QCOL:2 * QCOL], op=ALU.max)
                nc.vector.tensor_tensor(g2[:], ebuf[:, 2 * QCOL:3 * QCOL],
                                        ebuf[:, 3 * QCOL:4 * QCOL], op=ALU.max)
                nc.vector.tensor_tensor(g1[:], g1[:], g2[:], op=ALU.max)
                nc.vector.tensor_reduce(mm_t[:], g1[:],
                                        axis=mybir.AxisListType.X, op=ALU.max)

            def finish(m=m, ms=ms, last=last, ebuf=ebuf, mm_t=mm_t):
                lnm = stats.tile([P, 1], f32, tag="lnm", name="lnm")
                nc.scalar.activation(lnm[:], mm_t[:], ACTF.Ln)
                rec = stats.tile([P, 1], f32, tag="rec", name="rec")
                nc.vector.reciprocal(rec[:], mm_t[:])
                cvec = stats.tile([P, 1], f32, tag="cvec", name="cvec")
                nc.vector.tensor_tensor(cvec[:], lnm[:], rec[:], op=ALU.mult)
                for h in range(NQ):
                    hs = slice(h * QCOL, (h + 1) * QCOL)
                    ob = out_pool.tile([P, QCOL], f16, tag="ob")
                    if last and h % 2 == 1:
                        # ACT is idle after the final exps; split the tail
                        nc.scalar.activation(ob[:], ebuf[:, hs], ACTF.Copy,
                                             bias=0.0, scale=cvec[:])
                    else:
                        nc.vector.tensor_scalar(ob[:], ebuf[:, hs], cvec[:],
                                                None, op0=ALU.mult)
                    nc.sync.dma_start(out=out_ext[ms, hs], in_=ob[:])

            if last:
                if pending[0] is not None:   # M_TILES == 1 safety
                    pending[0]()
                    pending[0] = None
                finish()
            else:
                pending[0] = finish

        for m in range(M_TILES):
            run_mtile(m)

    nc.compile()
    return nc


_NC = None


def _get_nc():
    global _NC
    if _NC is None:
        _NC = _build()
    return _NC


def _prep(arr):
    """[N, 64] f32 levels -> [128, N] f16 feature-major, features duplicated."""
    t = np.ascontiguousarray(arr.T.astype(np.float16))       # [64, N]
    return np.ascontiguousarray(np.concatenate([t, t], axis=0))  # [128, N]


def make_in_maps(bandwidths, test_Xs, train_Xs):
    bandwidths = np.ascontiguousarray(bandwidths, dtype=np.float32)
    test_Xs = np.asarray(test_Xs, dtype=np.float32)
    train_Xs = np.asarray(train_Xs, dtype=np.float32)
    trainT = _prep(train_Xs)
    return [
        {
            "bandwidths": bandwidths,
            "testT": _prep(test_Xs[i * M_LOC:(i + 1) * M_LOC]),
            "trainT": trainT,
        }
        for i in range(N_CORES)
    ]


def kernel(bandwidths, test_Xs, train_Xs):
    nc = _get_nc()
    in_maps = make_in_maps(bandwidths, test_Xs, train_Xs)
    res = run_bass_kernel_spmd(nc, in_maps, core_ids=list(range(N_CORES)))
    return np.concatenate([np.asarray(r["out"]).astype(np.float32)
                           for r in res.results], axis=0)


if __name__ == "__main__":
    rng = np.random.default_rng(0)
    h = rng.uniform(0.05, 0.5, N_FEAT).astype(np.float32)
    t = rng.integers(0, 4, (N_TEST, N_FEAT)).astype(np.float32)
    s = rng.integers(0, 4, (N_TRAIN, N_FEAT)).astype(np.float32)
    out = kernel(bandwidths=h, test_Xs=t, train_Xs=s)
    print(out.shape, out.dtype)
